# revision 14
# baseline (speedup 1.0000x reference)
"""Trainium2 Bass kernel for fused multi-head attention (B=4, N=2048, D=384, h=8, dh=48).

Sharding: 32 (batch, head) pairs across 8 cores -> core c handles batch c//2 and
heads [4*(c%2), 4*(c%2)+4). Each core computes a *partial* output projection
(its 4 heads' contribution to out @ Wproj); the host sums the two partials per
batch and adds bproj.

Per-core dataflow (transposed layout, no PE transposes):
  xT   [384, 2048]  = x^T  (host, bf16)
  QT/KT[256, 2048]  = W^T @ xT   (4 heads, dh padded 48->64 so sim lhsT/rhs
                      partition offsets stay 32-aligned)
  V'   [2048, 196]  = x @ Wv packed 49 cols/head: 48 v-dims + a ones column at
                      col h*49+48 (gpsimd memset) so the PV matmul accumulates
                      the softmax denominator Z for free.
  per (head, q-half, key-chunk):
      simT[k,q] = KT_h^T @ QT_h   (PSUM, K=64)
      E = exp(simT)               (scalar ACT, psum->sbuf bf16; scores ~N(0,1)
                                   so no max subtraction needed)
      acc[o:o+49, q] += V'_h^T @ E  (PSUM accumulate; row o+48 = Z)
  OT_h = acc_h * (1/Z)            (DVE recip + gpsimd partition broadcast +
                                   DVE multiply), bf16; done per-head as soon
                                   as that head's 16 key chunks finish
  y    [2048, 384]  = sum_heads OT^T @ Wproj  (partial; f32, DMA'd per chunk)

Perf notes (from baseline trace): the scalar engine's exp stream (~956ns per
[128,1024] tile) and the PE matmul stream (~852ns/iter at full clock) are
nearly balanced; the PE only reaches its top p-state when it executes without
idle gaps, so V-chunk / QK(p1) / proj matmuls are interleaved into the
attention loop as filler instead of running as separate phases, and the scalar
engine runs exp ONLY (all copies on DVE/gpsimd).
"""

import os

os.environ.pop("JAX_PLATFORMS", None)  # the bass PJRT path needs the axon platform

import numpy as np
import ml_dtypes

import concourse.mybir as mybir
import concourse.tile as tile
from concourse import bacc
from concourse.bass_utils import run_bass_kernel_spmd

BF16 = ml_dtypes.bfloat16

# problem shapes (hardcoded per contract)
B, N, D = 4, 2048, 384
H, DH = 8, 48
SCALE = DH**-0.5
N_CORES = 8
HP = 4  # heads per core
DHP = 64  # padded head dim for Q/K (partition offsets must be 32-aligned)
VW = DH + 1  # 49: packed V width per head (48 dims + Z ones-column)
ZO = 32  # Z column offset within a head's block: engine reads of the Z row
# need a 32-aligned partition start, so v-dims occupy cols [0,32)+[33,49)
P = 128
NKC = N // P  # 16 key-row chunks

LAST_EXEC_NS = None
_CACHE = {}


def _build_bass():
    f32 = mybir.dt.float32
    bf16 = mybir.dt.bfloat16
    EXP = mybir.ActivationFunctionType.Exp

    nc = bacc.Bacc("TRN2", target_bir_lowering=False, debug=False, num_devices=N_CORES)
    xbT = nc.dram_tensor("xbT", [D, N], bf16, kind="ExternalInput").ap()
    # wall cols: [wk 0:256 | wq 256:512 | wv 512:708]
    wall = nc.dram_tensor("wall", [D, 2 * HP * DHP + HP * VW], bf16, kind="ExternalInput").ap()
    wpj = nc.dram_tensor("wpj", [2, P, D], bf16, kind="ExternalInput").ap()
    y = nc.dram_tensor("y", [N, D], f32, kind="ExternalOutput").ap()
    WC = 2 * HP * DHP + HP * VW  # 708

    with tile.TileContext(nc) as tc:
        with (
            tc.tile_pool(name="const", bufs=1) as cpool,
            tc.tile_pool(name="epool", bufs=4) as epool,
            tc.tile_pool(name="rpool", bufs=2) as rpool,
            tc.tile_pool(name="ysb", bufs=4) as ypool,
            tc.tile_pool(name="simps", bufs=2, space="PSUM") as simps,
            tc.tile_pool(name="accps", bufs=1, space="PSUM") as accps,
            tc.tile_pool(name="filps", bufs=2, space="PSUM") as filps,
        ):
            # ---- input DMA ----
            # Weights first on the sync HWDGE queue (first matmuls need the
            # whole W); xT halves split across sync/scalar/vector queues so
            # the first QK j-chunks unblock early. Scalar engine issues DMA
            # only here in phase A (it must be exp-only later).
            W = [cpool.tile([P, WC], bf16, name=f"W{i}", tag=f"W{i}") for i in range(3)]
            for i in range(3):
                nc.sync.dma_start(out=W[i][:], in_=wall[i * P : (i + 1) * P, :])
            xT = [cpool.tile([P, N], bf16, name=f"xT{i}", tag=f"xT{i}") for i in range(3)]
            for hf in range(2):
                for i in range(3):
                    eng = (nc.scalar, nc.gpsimd, nc.scalar)[i]
                    eng.dma_start(
                        out=xT[i][:, hf * (N // 2) : (hf + 1) * (N // 2)],
                        in_=xbT[i * P : (i + 1) * P, hf * (N // 2) : (hf + 1) * (N // 2)],
                    )
            wpj_sb = []
            for p in range(2):
                t = cpool.tile([P, D], bf16, name=f"wpj{p}", tag=f"wpj{p}")
                nc.gpsimd.dma_start(out=t[:], in_=wpj[p])
                wpj_sb.append(t)

            # Preload the Exp activation table while weights stream in, so the
            # first real exp doesn't pay the ~1.3us table load.
            warm = rpool.tile([1, 2], f32, name="warm", tag="warm")
            nc.scalar.activation(warm[:], W[0][0:1, 0:2], EXP)

            QT = [cpool.tile([P, N], bf16, name=f"QT{p}", tag=f"QT{p}") for p in range(2)]
            KT = [cpool.tile([P, N], bf16, name=f"KT{p}", tag=f"KT{p}") for p in range(2)]
            V = [cpool.tile([P, HP * VW], bf16, name=f"V{i}", tag=f"V{i}") for i in range(NKC)]
            OT = [cpool.tile([P, N], bf16, name=f"OT{p}", tag=f"OT{p}") for p in range(2)]
            # rows [hh*64+49, hh*64+64) of OT are never written (heads are
            # packed 49 wide) but proj's lhsT reads all 128 partitions; zero
            # them so stale NaN bit patterns can't poison 0-weight products.
            # memset offset must be 32-aligned; rows [o+32, o+49) get
            # overwritten by norm later.
            for p in range(2):
                for hh in range(2):
                    nc.gpsimd.memset(OT[p][hh * DHP + 32 : (hh + 1) * DHP, :], 0.0)

            def qk_group(p, grp, j, eng):
                # one [128, 512] chunk of QT/KT pair p (grp 0 = K, 1 = Q)
                ps = filps.tile([P, 512], f32, name="qkps", tag="fil")
                for dk in range(3):
                    nc.tensor.matmul(
                        ps[:],
                        lhsT=W[dk][:, grp * 256 + p * P : grp * 256 + (p + 1) * P],
                        rhs=xT[dk][:, j * 512 : (j + 1) * 512],
                        start=(dk == 0),
                        stop=(dk == 2),
                    )
                dst = (KT, QT)[grp]
                if eng is nc.scalar:
                    eng.copy(dst[p][:, j * 512 : (j + 1) * 512], ps[:])
                else:
                    eng.tensor_copy(dst[p][:, j * 512 : (j + 1) * 512], ps[:])

            def v_chunk(i, eng):
                ps = filps.tile([P, 512], f32, name="vps", tag="fil")
                for dk in range(3):
                    nc.tensor.matmul(
                        ps[:, : HP * VW],
                        lhsT=xT[dk][:, i * P : (i + 1) * P],
                        rhs=W[dk][:, 2 * HP * DHP :],
                        start=(dk == 0),
                        stop=(dk == 2),
                    )
                t = V[i]
                if eng is nc.scalar:
                    eng.copy(t[:], ps[:, : HP * VW])
                else:
                    eng.tensor_copy(t[:], ps[:, : HP * VW])
                # ones (Z) column of each head block, at col h*49+ZO
                zcols = t[:].rearrange("p (h c) -> p h c", c=VW)[:, :, ZO : ZO + 1]
                nc.gpsimd.memset(zcols, 1.0)

            def proj_chunk(mc, eng):
                yp = filps.tile([P, 512], f32, name="yp", tag="fil")
                for p in range(2):
                    nc.tensor.matmul(
                        yp[:, :D],
                        lhsT=OT[p][:, mc * P : (mc + 1) * P],
                        rhs=wpj_sb[p][:],
                        start=(p == 0),
                        stop=(p == 1),
                    )
                ys = ypool.tile([P, D], f32, name="ys", tag="ys")
                if eng is nc.scalar:
                    eng.copy(ys[:], yp[:, :D])
                else:
                    eng.tensor_copy(ys[:], yp[:, :D])
                (nc.sync if mc % 2 == 0 else nc.gpsimd).dma_start(
                    out=y[mc * P : (mc + 1) * P, :], in_=ys[:]
                )

            def norm(p, qh, hh, acc, nslice=1):
                # OT rows [o, o+49) = acc rows / Z (Z = acc row o+ZO)
                o = hh * DHP
                w = 1024 // nslice
                for s in range(nslice):
                    c0 = s * w
                    zr = rpool.tile([1, w], f32, name="zr", tag="zr")
                    nc.vector.tensor_copy(zr[:], acc[o + ZO : o + ZO + 1, c0 : c0 + w])
                    r = rpool.tile([1, w], f32, name="r", tag="r")
                    nc.vector.reciprocal_approx_fast(r[:], zr[:])
                    R = rpool.tile([VW, w], f32, name="R", tag="R")
                    nc.gpsimd.partition_broadcast(R[:], r[:], channels=VW)
                    nc.vector.tensor_mul(
                        OT[p][o : o + VW, qh * 1024 + c0 : qh * 1024 + c0 + w],
                        acc[o : o + VW, c0 : c0 + w],
                        R[:],
                    )

            # ---- phase A: QK(p0) + first V chunks ----
            # copies alternate scalar/DVE here (no exp running yet)
            for j in range(4):
                qk_group(0, 0, j, nc.scalar if j % 2 == 0 else nc.vector)
                qk_group(0, 1, j, nc.vector if j % 2 == 0 else nc.scalar)
            v_chunk(0, nc.scalar)
            v_chunk(1, nc.vector)

            # ---- phase B: attention blocks with interleaved filler ----
            # block order: (p0,qh0) fills V[2..15], (p0,qh1) fills QK(p1),
            # (p1,qh0) no filler, (p1,qh1) fills proj qh0; tail = proj qh1.
            def attention_block(p, qh, fillers, want_fn=None, last=False):
                acc = accps.tile([P, 1024], f32, name="acc", tag="acc")
                nf = len(fillers)
                if want_fn is None:
                    want_fn = lambda it: (it + 1) * nf // 32
                fi = 0
                for hh in range(2):
                    h = p * 2 + hh
                    o = hh * DHP
                    for kc in range(NKC):
                        it = hh * NKC + kc
                        sp = simps.tile([P, 1024], f32, name="sim", tag="sim")
                        for j in range(2):
                            nc.tensor.matmul(
                                sp[:, j * 512 : (j + 1) * 512],
                                lhsT=KT[p][o : o + DHP, kc * P : (kc + 1) * P],
                                rhs=QT[p][
                                    o : o + DHP,
                                    qh * 1024 + j * 512 : qh * 1024 + (j + 1) * 512,
                                ],
                                start=True,
                                stop=True,
                            )
                        e = epool.tile([P, 1024], bf16, name="E", tag="E")
                        nc.scalar.activation(e[:], sp[:], EXP)
                        for j in range(2):
                            nc.tensor.matmul(
                                acc[o : o + VW, j * 512 : (j + 1) * 512],
                                lhsT=V[kc][:, h * VW : (h + 1) * VW],
                                rhs=e[:, j * 512 : (j + 1) * 512],
                                start=(kc == 0),
                                stop=(kc == NKC - 1),
                            )
                        # interleave filler matmuls to keep the PE dense
                        want = min(nf, want_fn(it))
                        while fi < want:
                            fillers[fi]()
                            fi += 1
                    # this head's acc rows are final -> normalize now
                    norm(p, qh, hh, acc, nslice=(2 if (last and hh == 1) else 1))

            # b1: V[kc] must be emitted ~2 iterations before PV(kc) consumes it
            attention_block(
                0,
                0,
                [(lambda i=i: v_chunk(i, nc.vector)) for i in range(2, NKC)],
                want_fn=lambda it: it + 1,
            )
            attention_block(
                0,
                1,
                [
                    (lambda g=g, j=j: qk_group(1, g, j, nc.vector))
                    for j in range(4)
                    for g in range(2)
                ],
            )
            attention_block(1, 0, [])
            attention_block(
                1,
                1,
                [(lambda mc=mc: proj_chunk(mc, nc.vector)) for mc in range(8)],
                last=True,
            )
            for mc in range(8, NKC):
                proj_chunk(mc, nc.scalar if mc % 2 else nc.vector)

    nc.compile()
    return nc


def _prep_core_inputs(x, Wqkv, Wproj, core):
    b, hg = core // 2, core % 2
    heads = [hg * HP + i for i in range(HP)]
    xbT = np.ascontiguousarray(x[b].astype(BF16).T)
    wall = np.zeros((D, 2 * HP * DHP + HP * VW), np.float32)
    wpj = np.zeros((2, P, D), np.float32)
    C = H * DH
    for i, h in enumerate(heads):
        # wk at cols [i*64, i*64+48), wq at [256 + i*64, ...), wv packed 49
        wall[:, i * DHP : i * DHP + DH] = Wqkv[:, C + h * DH : C + (h + 1) * DH]
        wall[:, HP * DHP + i * DHP : HP * DHP + i * DHP + DH] = (
            Wqkv[:, h * DH : (h + 1) * DH] * SCALE
        )
        # wv packed 49/head with a zero (-> ones) column at ZO
        wv_h = Wqkv[:, 2 * C + h * DH : 2 * C + (h + 1) * DH]
        v0 = 2 * HP * DHP + i * VW
        wall[:, v0 : v0 + ZO] = wv_h[:, :ZO]
        wall[:, v0 + ZO + 1 : v0 + VW] = wv_h[:, ZO:]
        # wpj rows mirror the packed layout (Z/pad rows stay 0)
        wpj_h = Wproj[h * DH : (h + 1) * DH, :]
        o = (i % 2) * DHP
        wpj[i // 2, o : o + ZO, :] = wpj_h[:ZO, :]
        wpj[i // 2, o + ZO + 1 : o + VW, :] = wpj_h[ZO:, :]
    return {
        "xbT": xbT,
        "wall": wall.astype(BF16),
        "wpj": wpj.astype(BF16),
    }


def kernel(x, Wqkv, Wproj, bproj):
    global LAST_EXEC_NS
    if "nc" not in _CACHE:
        _CACHE["nc"] = _build_bass()
    nc = _CACHE["nc"]
    in_maps = [_prep_core_inputs(x, Wqkv, Wproj, c) for c in range(N_CORES)]
    try:
        res = run_bass_kernel_spmd(nc, in_maps, core_ids=list(range(N_CORES)))
    except Exception:
        res = run_bass_kernel_spmd(nc, in_maps, core_ids=list(range(N_CORES)))
    LAST_EXEC_NS = res.exec_time_ns
    out = np.empty((B, N, D), np.float32)
    for b in range(B):
        out[b] = res.results[2 * b]["y"] + res.results[2 * b + 1]["y"]
    out += bproj.astype(np.float32)[None, None, :]
    return out
